# revision 1
# baseline (speedup 1.0000x reference)
"""Fused Conv3x3+BN+LeakyReLU -> QKV -> spatial self-attention -> residual+LN+LeakyReLU
Trainium2 Bass kernel, data-parallel over batch on 8 NeuronCores.

Per-core layout (4 samples): features are "c-major" [channel(2x128 part), pixel].
Conv = 9-tap matmul accumulation over a zero-padded c-major input (f32r).
BatchNorm statistics are AllReduced across the 8 cores (batch is sharded).
Attention per sample in bf16; softmax skips max-subtraction (scores are O(5));
the softmax denominator is computed with a ones-matmul that simultaneously
reduces over partitions and broadcasts the result to all of them.
LayerNorm tail is per-sample so it overlaps the next sample's attention.
"""
import sys
import numpy as np

sys.path.insert(0, "/opt/trn_rl_repo")

N_CORES = 8
S = 4            # samples per core
H = W = 32
C = 256
NPIX = S * H * W            # 4096 pixels per core
HP = H + 2                  # padded spatial extent
ALPHA = 0.3
BN_EPS = 1e-3
LN_EPS = 1e-3

_CACHE = {}


def _build(repeat=1):
    import concourse.bacc as bacc
    import concourse.tile as tile
    from concourse import bass_isa
    from concourse.tile import add_dep_helper
    import concourse.mybir as mybir
    from concourse.masks import make_identity

    F32 = mybir.dt.float32
    F32R = mybir.dt.float32r
    BF16 = mybir.dt.bfloat16
    AF = mybir.ActivationFunctionType
    OP = mybir.AluOpType

    nc = bacc.Bacc("TRN2", target_bir_lowering=False, debug=False,
                   num_devices=N_CORES)

    x_s = nc.declare_dram_parameter("x_s", [NPIX, C], F32, isOutput=False)
    w_cbl = nc.declare_dram_parameter("w_cbl", [3, 3, C, C], F32, isOutput=False)
    b_cbl = nc.declare_dram_parameter("b_cbl", [C], F32, isOutput=False)
    bn_gamma = nc.declare_dram_parameter("bn_gamma", [C], F32, isOutput=False)
    bn_beta = nc.declare_dram_parameter("bn_beta", [C], F32, isOutput=False)
    wq = nc.declare_dram_parameter("wq", [C, C], F32, isOutput=False)
    bq = nc.declare_dram_parameter("bq", [C], F32, isOutput=False)
    wk = nc.declare_dram_parameter("wk", [C, C], F32, isOutput=False)
    bk = nc.declare_dram_parameter("bk", [C], F32, isOutput=False)
    wv = nc.declare_dram_parameter("wv", [C, C], F32, isOutput=False)
    bv = nc.declare_dram_parameter("bv", [C], F32, isOutput=False)
    ln_gamma = nc.declare_dram_parameter("ln_gamma", [H * W, C], F32, isOutput=False)
    ln_beta = nc.declare_dram_parameter("ln_beta", [H * W, C], F32, isOutput=False)
    y_s = nc.declare_dram_parameter("y_s", [NPIX, C], F32, isOutput=True)

    def ecopy(i, out, in_):
        if i % 2 == 0:
            nc.vector.tensor_copy(out, in_)
        else:
            nc.scalar.copy(out, in_)

    with tile.TileContext(nc) as tc:
        import contextlib
        est = contextlib.ExitStack()
        with est:
            persist = est.enter_context(tc.tile_pool(name="persist", bufs=1))
            pstat = est.enter_context(tc.tile_pool(name="pstat", bufs=1))
            dram = est.enter_context(tc.tile_pool(name="dram", bufs=1, space="DRAM"))

            # ---------- persistent constants ----------
            ident = persist.tile([128, 128], F32, tag="ident")
            make_identity(nc, ident[:])
            ident_r = persist.tile([128, 128], F32R, tag="ident_r")
            nc.vector.tensor_copy(ident_r[:], ident[:])
            ones_bf = persist.tile([128, 128], BF16, tag="ones_bf")
            nc.vector.memset(ones_bf[:], 1.0)
            ones1f = persist.tile([1, 128], F32, tag="ones1f")
            nc.vector.memset(ones1f[:], 1.0)
            ones1r = persist.tile([1, 128], F32R, tag="ones1r")
            nc.vector.tensor_copy(ones1r[:], ones1f[:])
            onescf = persist.tile([128, 1], F32, tag="onescf")
            nc.vector.memset(onescf[:], 1.0)
            onescr = persist.tile([128, 1], F32R, tag="onescr")
            nc.vector.tensor_copy(onescr[:], onescf[:])
            eps_sb = persist.tile([128, 1], F32, tag="eps_sb")
            nc.vector.memset(eps_sb[:], BN_EPS)

            pre = persist.tile([1, 4], F32, tag="pre")
            nc.vector.memset(pre[:], 1.0)

            def load_vec(handle, name):
                t = persist.tile([128, 2], F32, tag="vec_" + name, name=name + "_sb")
                nc.gpsimd.dma_start(out=t[:], in_=handle.ap().rearrange("(g p) -> p g", g=2))
                return t

            bcbl_sb = load_vec(b_cbl, "bcbl")
            bng_sb = load_vec(bn_gamma, "bng")
            bnb_sb = load_vec(bn_beta, "bnb")
            bq_sb = load_vec(bq, "bq")
            bk_sb = load_vec(bk, "bk")
            bv_sb = load_vec(bv, "bv")
            bqs_sb = persist.tile([128, 2], F32, tag="bqs")
            nc.gpsimd.tensor_scalar_mul(bqs_sb[:], bq_sb[:], 1.0 / 16.0)

            wqkv_r = {}
            cT0 = persist.tile([128, NPIX], F32R, tag="cT0")
            cT1 = persist.tile([128, NPIX], F32R, tag="cT1")
            cTs = [cT0, cT1]
            lng = persist.tile([128, 2, 1024], F32, tag="lng")
            lnb = persist.tile([128, 2, 1024], F32, tag="lnb")

            def phase_body():
                # =========== conv phase ===========
                with tc.tile_pool(name="convp", bufs=1) as convp, \
                     tc.tile_pool(name="setup", bufs=1) as setup, \
                     tc.tile_pool(name="cvps", bufs=4, space="PSUM") as cvps, \
                     tc.tile_pool(name="tpps", bufs=4, space="PSUM") as tpps:
                    xpads = [convp.tile([128, 2, HP, HP], F32R, tag=f"xpad{s}",
                                        name=f"xpad{s}") for s in range(S)]
                    wc_r = convp.tile([128, 2, 9, C], F32R, tag="wc_r")
                    convraw = convp.tile([128, 2, NPIX], F32, tag="convraw")
                    cstat = pstat.tile([128, 2, 8, 6], F32, tag="cstat")

                    # ---- input DMA + transpose to padded c-major, per sample ----
                    zb = setup.tile([128, HP], F32, tag="zb")
                    nc.vector.memset(zb[:], 0.0)
                    xr = x_s.ap().rearrange("(t p) c -> p t c", p=128)
                    xpixs = []
                    wsts = []
                    wdram = w_cbl.ap().rearrange("a b (g p) d -> p (a b) g d", g=2)
                    # order: x0, then both conv-weight halves (they gate the
                    # first conv matmul), then the remaining x chunks.
                    for s in range(S):
                        xpix = setup.tile([128, 8, C], F32, tag=f"xpix{s}",
                                          name=f"xpix{s}")
                        nc.sync.dma_start(out=xpix[:], in_=xr[:, s * 8:(s + 1) * 8, :])
                        xpixs.append(xpix)
                        if s == 0:
                            for ch in range(2):
                                wstc = setup.tile([128, 9, C], F32, tag=f"wst{ch}",
                                                  name=f"wst{ch}")
                                nc.scalar.dma_start(out=wstc[:],
                                                    in_=wdram[:, :, ch, :])
                                wsts.append(wstc)
                    for s in range(S):
                        xpad = xpads[s]
                        for ch in range(2):
                            k = s * 2 + ch
                            ecopy(k, xpad[:, ch, 0, :], zb[:])
                            ecopy(k + 1, xpad[:, ch, HP - 1, :], zb[:])
                            ecopy(k, xpad[:, ch, :, 0], zb[:])
                            ecopy(k + 1, xpad[:, ch, :, HP - 1], zb[:])
                        for tb in range(8):
                            r0 = tb * 4
                            for ch in range(2):
                                tp = tpps.tile([128, 128], F32, tag="tp")
                                nc.tensor.transpose(
                                    tp[:], xpixs[s][:, tb, ch * 128:(ch + 1) * 128],
                                    ident[:])
                                ecopy(tb * 2 + ch,
                                      xpad[:, ch, 1 + r0:5 + r0, 1:1 + W],
                                      tp[:].rearrange("p (a b) -> p a b", b=W))

                    for ch in range(2):
                        nc.vector.tensor_copy(wc_r[:, ch, :, :], wsts[ch][:])
                    # preload ACT spline tables while ACT is otherwise idle
                    for fn in (AF.Identity, AF.Sqrt, AF.Exp, AF.Prelu):
                        nc.scalar.activation(pre[:, 2:3], pre[:, 0:1], fn, alpha=ALPHA)

                    mvc = pstat.tile([128, 2, 2], F32, tag="mvc")
                    s12 = pstat.tile([128, 4], F32, tag="s12")
                    msq = pstat.tile([128, 2], F32, tag="msq")
                    cc_outs = []
                    cc_insts = []
                    lns0 = setup.tile([128, 8, C], F32, tag="xpix0", name="lns0")
                    lns1 = setup.tile([128, 8, C], F32, tag="xpix1", name="lns1")
                    nc.scalar.dma_start(
                        out=lns0[:],
                        in_=ln_gamma.ap().rearrange("(t p) c -> p t c", p=128))
                    nc.scalar.dma_start(
                        out=lns1[:],
                        in_=ln_beta.ap().rearrange("(t p) c -> p t c", p=128))
                # ---- conv matmuls ----
                    for dh in range(2):
                        for cp in range(4):
                            pss = [cvps.tile([128, 512], F32, tag="cv",
                                             name=f"cv_{dh}_{cp}_{h}") for h in range(2)]
                            for it in range(9):
                                ky, kx = divmod(it, 3)
                                for ch in range(2):
                                    lhsT = wc_r[:, ch, it, dh * 128:(dh + 1) * 128]
                                    first = (it == 0 and ch == 0)
                                    last = (it == 8 and ch == 1)
                                    for hf in range(2):
                                        chunk = cp * 2 + hf
                                        s, rbh = divmod(chunk, 2)
                                        rb = rbh * 16
                                        rhs = xpads[s][:, ch, rb + ky:rb + ky + 16,
                                                       kx:kx + W]
                                        nc.tensor.matmul(pss[hf][:], lhsT, rhs,
                                                         start=first, stop=last)
                            for hf in range(2):
                                chunk = cp * 2 + hf
                                sl = slice(chunk * 512, (chunk + 1) * 512)
                                nc.scalar.activation(
                                    convraw[:, dh, sl], pss[hf][:], AF.Identity,
                                    bias=bcbl_sb[:, dh:dh + 1], scale=1.0)
                                nc.vector.bn_stats(out=cstat[:, dh, chunk, :],
                                                   in_=convraw[:, dh, sl])
                        # aggregate this half while the other half's conv runs
                        nc.vector.bn_aggr(out=mvc[:, dh, :], in_=cstat[:, dh, :, :])
                        nc.vector.tensor_scalar_mul(
                            s12[:, dh:dh + 1], mvc[:, dh, 0:1], float(NPIX))
                        nc.vector.tensor_mul(
                            msq[:, dh:dh + 1], mvc[:, dh, 0:1], mvc[:, dh, 0:1])
                        nc.vector.tensor_add(
                            msq[:, dh:dh + 1], msq[:, dh:dh + 1], mvc[:, dh, 1:2])
                        nc.vector.tensor_scalar_mul(
                            s12[:, 2 + dh:3 + dh], msq[:, dh:dh + 1], float(NPIX))
                        # launch this half's stats exchange; dh0's hides under
                        # dh1's conv matmuls
                        cc_in_d = dram.tile([128, 2], F32, tag=f"cc_in{dh}",
                                            name=f"cc_in{dh}")
                        cc_out_d = dram.tile([N_CORES * 128, 2], F32,
                                             tag=f"cc_out{dh}", name=f"cc_out{dh}")
                        nc.sync.dma_start(out=cc_in_d[:], in_=s12[:, dh:dh + 3:2])
                        cc_i = nc.gpsimd.collective_compute(
                            "AllGather", OP.bypass,
                            replica_groups=[list(range(N_CORES))],
                            ins=[cc_in_d.opt()], outs=[cc_out_d.opt()])
                        cc_outs.append(cc_out_d)
                        cc_insts.append(cc_i)

                    # ---- setup work placed here so it fills the AllReduce window ----
                    for t in range(8):
                        for ch in range(2):
                            for li, (lnst, dst) in enumerate(
                                    ((lns0, lng), (lns1, lnb))):
                                tp = tpps.tile([128, 128], F32, tag="tp")
                                tpi = nc.tensor.transpose(
                                    tp[:], lnst[:, t, ch * 128:(ch + 1) * 128], ident[:])
                                # pin into the second collective's wait window
                                add_dep_helper(tpi.ins, cc_insts[-1].ins, sync=False,
                                               reason="fill collective wait")
                                ecopy(t * 2 + ch + li,
                                      dst[:, ch, t * 128:(t + 1) * 128], tp[:])
                    wqs = setup.tile([128, 2, C], F32, tag="wqs")
                    for handle, name in ((wq, "wq"), (wk, "wk"), (wv, "wv")):
                        wr = persist.tile([128, 2, C], F32R, tag="wr_" + name,
                                          name=name + "_r")
                        nc.sync.dma_start(
                            out=wqs[:], in_=handle.ap().rearrange("(g p) d -> p g d", g=2))
                        nc.scalar.copy(wr[:], wqs[:])
                        wqkv_r[name] = wr

                    # ---- per-half: gather partials, finish stats, apply BN ----
                    NTOT = float(N_CORES * NPIX)
                    for dh in range(2):
                        g8d = pstat.tile([128, 2, N_CORES], F32, tag=f"g8_{dh}",
                                         name=f"g8_{dh}")
                        nc.sync.dma_start(
                            out=g8d[:],
                            in_=cc_outs[dh].rearrange("(k p) c -> p c k", k=N_CORES))
                        g2 = pstat.tile([128, 2], F32, tag=f"g2_{dh}",
                                        name=f"g2_{dh}")
                        nc.vector.reduce_sum(g2[:], g8d[:],
                                             axis=mybir.AxisListType.X)
                        gws = pstat.tile([128, 4], F32, tag=f"gws_{dh}",
                                         name=f"gws_{dh}")
                        nc.vector.tensor_scalar_mul(gws[:, 0:1], g2[:, 0:1], 1.0 / NTOT)
                        nc.vector.tensor_scalar_mul(gws[:, 1:2], g2[:, 1:2], 1.0 / NTOT)
                        nc.vector.tensor_mul(gws[:, 2:3], gws[:, 0:1], gws[:, 0:1])
                        nc.vector.tensor_sub(gws[:, 1:2], gws[:, 1:2], gws[:, 2:3])
                        nc.scalar.activation(gws[:, 3:4], gws[:, 1:2], AF.Sqrt,
                                             bias=eps_sb[:])
                        nc.vector.reciprocal(gws[:, 2:3], gws[:, 3:4])
                        scsh = pstat.tile([128, 2], F32, tag=f"scsh_{dh}",
                                          name=f"scsh_{dh}")
                        nc.vector.tensor_mul(scsh[:, 0:1], bng_sb[:, dh:dh + 1],
                                             gws[:, 2:3])
                        nc.vector.tensor_mul(scsh[:, 1:2], gws[:, 0:1], scsh[:, 0:1])
                        nc.vector.tensor_sub(scsh[:, 1:2], bnb_sb[:, dh:dh + 1],
                                             scsh[:, 1:2])
                        nc.scalar.activation(
                            cTs[dh][:], convraw[:, dh, :], AF.Prelu,
                            bias=scsh[:, 1:2], scale=scsh[:, 0:1], alpha=ALPHA)

                # =========== attention phase ===========
                with tc.tile_pool(name="attp", bufs=1) as attp, \
                     tc.tile_pool(name="ypool", bufs=1) as ypool:
                    qbf = attp.tile([128, 2, NPIX], BF16, tag="qbf")
                    kbf = attp.tile([128, 2, NPIX], BF16, tag="kbf")
                    v2bf = attp.tile([128, 32, C], BF16, tag="v2bf")

                    with tc.tile_pool(name="qkps", bufs=4, space="PSUM") as qkps:
                        for dh in range(2):
                            for chunk in range(8):
                                sl = slice(chunk * 512, (chunk + 1) * 512)
                                psq = qkps.tile([128, 512], F32, tag="qk")
                                psk = qkps.tile([128, 512], F32, tag="qk")
                                for ch in range(2):
                                    nc.tensor.matmul(
                                        psq[:], wqkv_r["wq"][:, ch, dh * 128:(dh + 1) * 128],
                                        cTs[ch][:, sl], start=(ch == 0), stop=(ch == 1))
                                    nc.tensor.matmul(
                                        psk[:], wqkv_r["wk"][:, ch, dh * 128:(dh + 1) * 128],
                                        cTs[ch][:, sl], start=(ch == 0), stop=(ch == 1))
                                nc.scalar.activation(
                                    qbf[:, dh, sl], psq[:], AF.Identity,
                                    bias=bqs_sb[:, dh:dh + 1], scale=1.0 / 16.0)
                                nc.vector.tensor_scalar_add(
                                    kbf[:, dh, sl], psk[:], bk_sb[:, dh:dh + 1])
                        for jt32 in range(32):
                            psv = qkps.tile([128, 512], F32, tag="qk")
                            for ch in range(2):
                                nc.tensor.matmul(
                                    psv[:, 0:C], cTs[ch][:, jt32 * 128:(jt32 + 1) * 128],
                                    wqkv_r["wv"][:, ch, :], start=(ch == 0), stop=(ch == 1))
                            ecopy(jt32, v2bf[:, jt32, :], psv[:, 0:C])

                    # ---- per-sample attention + residual + LN + output ----
                    with tc.tile_pool(name="attps", bufs=6, space="PSUM") as attps, \
                         tc.tile_pool(name="tpo", bufs=2, space="PSUM") as tpo:
                        for s in range(S):
                            Es = []
                            for jt in range(8):
                                sps = attps.tile([128, 2, 512], F32, tag="sc2", bufs=2,
                                                 name=f"sc_{s}_{jt}")
                                for nh in range(2):
                                    for ch in range(2):
                                        nc.tensor.matmul(
                                            sps[:, nh, :],
                                            kbf[:, ch, s * 1024 + jt * 128:s * 1024 + (jt + 1) * 128],
                                            qbf[:, ch, s * 1024 + nh * 512:s * 1024 + (nh + 1) * 512],
                                            start=(ch == 0), stop=(ch == 1))
                                E = attp.tile([128, 1024], BF16, tag="E", bufs=14,
                                              name=f"E_{s}_{jt}")
                                nc.scalar.activation(
                                    E[:], sps[:].rearrange("p a b -> p (a b)"), AF.Exp)
                                Es.append(E)
                            # Z: reduce over j-partitions AND broadcast to 128 rows
                            zr = ypool.tile([128, 1024], F32, tag="zr", bufs=2,
                                            name=f"zr_{s}")
                            for nh in range(2):
                                zps = attps.tile([128, 512], F32, tag="zat", bufs=2,
                                                 name=f"z_{s}_{nh}")
                                for jt in range(8):
                                    nc.tensor.matmul(
                                        zps[:], ones_bf[:],
                                        Es[jt][:, nh * 512:(nh + 1) * 512],
                                        start=(jt == 0), stop=(jt == 7))
                                nc.vector.reciprocal(
                                    zr[:, nh * 512:(nh + 1) * 512], zps[:])
                            ys = ypool.tile([128, 2, 1024], F32, tag="y", bufs=2,
                                            name=f"y_{s}")
                            lstat = pstat.tile([128, 2, 2, 6], F32, tag="lstat", bufs=2,
                                               name=f"lstat_{s}")
                            for ch in range(2):
                                attn = ypool.tile([128, 1024], F32, tag="tmp", bufs=4,
                                                  name=f"attn_{s}_{ch}")
                                for nh in range(2):
                                    aps = attps.tile([128, 512], F32, tag="zat", bufs=2,
                                                     name=f"at_{s}_{ch}_{nh}")
                                    for jt in range(8):
                                        nc.tensor.matmul(
                                            aps[:],
                                            v2bf[:, s * 8 + jt, ch * 128:(ch + 1) * 128],
                                            Es[jt][:, nh * 512:(nh + 1) * 512],
                                            start=(jt == 0), stop=(jt == 7))
                                    nc.vector.tensor_mul(
                                        attn[:, nh * 512:(nh + 1) * 512], aps[:],
                                        zr[:, nh * 512:(nh + 1) * 512])
                                yadd = nc.vector.tensor_add if ch == 0 else nc.gpsimd.tensor_add
                                yadd(
                                    ys[:, ch, :], attn[:],
                                    cTs[ch][:, s * 1024:(s + 1) * 1024].bitcast(F32))
                                for b2 in range(2):
                                    nc.vector.bn_stats(
                                        out=lstat[:, ch, b2, :],
                                        in_=ys[:, ch, b2 * 512:(b2 + 1) * 512])

                            # per-sample LN scalars
                            lmv = pstat.tile([128, 2, 2], F32, tag="lmv", bufs=2,
                                             name=f"lmv_{s}")
                            for ch in range(2):
                                nc.vector.bn_aggr(out=lmv[:, ch, :], in_=lstat[:, ch, :, :])
                            SCs = pstat.tile([128, 4], F32, tag="SCs", bufs=2,
                                             name=f"SCs_{s}")
                            lms = pstat.tile([128, 2], F32, tag="lms", bufs=2,
                                             name=f"lms_{s}")
                            nc.vector.tensor_mul(lms[:], lmv[:, :, 0], lmv[:, :, 0])
                            nc.vector.tensor_add(lms[:], lms[:], lmv[:, :, 1])
                            nc.vector.tensor_scalar_mul(SCs[:, 0:2], lmv[:, :, 0], 1024.0)
                            nc.vector.tensor_scalar_mul(SCs[:, 2:4], lms[:], 1024.0)
                            T128 = pstat.tile([128, 4], F32, tag="T128", bufs=2,
                                              name=f"T128_{s}")
                            nc.gpsimd.partition_all_reduce(
                                T128[:], SCs[:], channels=128,
                                reduce_op=bass_isa.ReduceOp.add)
                            NLN = float(H * W * C)
                            wk4 = pstat.tile([128, 4], F32, tag="wk4", bufs=2,
                                             name=f"wk4_{s}")
                            # wk4 cols: 0=mean 1=E[y^2] 2=scratch 3=sd
                            nc.vector.tensor_add(wk4[:, 0:2], T128[:, 0:4:2],
                                                 T128[:, 1:4:2])
                            nc.vector.tensor_scalar_mul(wk4[:, 0:2], wk4[:, 0:2],
                                                        1.0 / NLN)
                            nc.vector.tensor_mul(wk4[:, 2:3], wk4[:, 0:1], wk4[:, 0:1])
                            nc.vector.tensor_sub(wk4[:, 1:2], wk4[:, 1:2], wk4[:, 2:3])
                            nc.scalar.activation(wk4[:, 3:4], wk4[:, 1:2], AF.Sqrt,
                                                 bias=eps_sb[:])
                            musd = pstat.tile([128, 2], F32, tag="musd", bufs=2,
                                              name=f"musd_{s}")
                            # musd: col0 = istd, col1 = mean
                            nc.vector.reciprocal(musd[:, 0:1], wk4[:, 3:4])
                            nc.vector.tensor_copy(musd[:, 1:2], wk4[:, 0:1])
                            s2t = pstat.tile([128, 2], F32, tag="s2t", bufs=2,
                                             name=f"s2t_{s}")
                            for ch in range(2):
                                nc.vector.tensor_sub(
                                    s2t[:, ch:ch + 1], bv_sb[:, ch:ch + 1], musd[:, 1:2])
                                nc.vector.tensor_mul(
                                    s2t[:, ch:ch + 1], s2t[:, ch:ch + 1], musd[:, 0:1])

                            outst = attp.tile([128, 8, C], F32, tag="outst", bufs=2,
                                              name=f"outst_{s}")
                            for ch in range(2):
                                yn = ypool.tile([128, 1024], F32, tag="tmp", bufs=4,
                                                name=f"yn_{s}_{ch}")
                                nc.vector.tensor_scalar(
                                    out=yn[:], in0=ys[:, ch, :],
                                    scalar1=musd[:, 0:1], scalar2=s2t[:, ch:ch + 1],
                                    op0=OP.mult, op1=OP.add)
                                yg = ypool.tile([128, 1024], F32, tag="tmp", bufs=4,
                                                name=f"yg_{s}_{ch}")
                                geng = nc.vector if ch == 0 else nc.gpsimd
                                geng.tensor_mul(yg[:], yn[:], lng[:, ch, :])
                                geng.tensor_add(yg[:], yg[:], lnb[:, ch, :])
                                yo = ypool.tile([128, 1024], F32R, tag="yo", bufs=2,
                                                name=f"yo_{s}_{ch}")
                                nc.scalar.activation(yo[:], yg[:], AF.Prelu, alpha=ALPHA)
                                for t in range(8):
                                    tp = tpo.tile([128, 128], F32R, tag="tpo")
                                    nc.tensor.transpose(
                                        tp[:], yo[:, t * 128:(t + 1) * 128].bitcast(F32R),
                                        ident_r[:])
                                    ecopy(t, outst[:, t, ch * 128:(ch + 1) * 128], tp[:])
                            nc.sync.dma_start(
                                out=y_s.ap()[s * 1024:(s + 1) * 1024, :].rearrange(
                                    "(t p) c -> p t c", p=128),
                                in_=outst[:])

            for _rep in range(repeat):
                phase_body()

    nc.compile()
    return nc


def _get_nc(repeat=1):
    key = ("nc", repeat)
    if key not in _CACHE:
        _CACHE[key] = _build(repeat)
    return _CACHE[key]


def _make_in_maps(inputs):
    x = np.ascontiguousarray(inputs["x"], dtype=np.float32)
    shared = {k: np.ascontiguousarray(inputs[k], np.float32)
              for k in ("w_cbl", "b_cbl", "bn_gamma", "bn_beta", "wq", "bq",
                        "wk", "bk", "wv", "bv")}
    shared["ln_gamma"] = np.ascontiguousarray(
        inputs["ln_gamma"], np.float32).reshape(H * W, C)
    shared["ln_beta"] = np.ascontiguousarray(
        inputs["ln_beta"], np.float32).reshape(H * W, C)
    in_maps = []
    for i in range(N_CORES):
        m = dict(shared)
        m["x_s"] = x[i * S:(i + 1) * S].reshape(NPIX, C)
        in_maps.append(m)
    return in_maps


def kernel(**inputs):
    from concourse.bass_utils import run_bass_kernel_spmd

    nc = _get_nc()
    in_maps = _make_in_maps(inputs)
    res = run_bass_kernel_spmd(nc, in_maps, list(range(N_CORES)))
    _CACHE["last_results"] = res
    out = np.empty((N_CORES * S, H, W, C), np.float32)
    for i in range(N_CORES):
        out[i * S:(i + 1) * S] = res.results[i]["y_s"].reshape(S, H, W, C)
    return out

